# revision 1
# baseline (speedup 1.0000x reference)
"""GCN (2-layer, symmetric-normalized A+I) on 8 Trainium2 NeuronCores.

Strategy (node-range sharded, one AllGather):
  - deg/dinv computed on host from edge_index (int preprocessing / "degrees").
  - Core k owns dst rows [k*R, (k+1)*R).  All per-edge scaling is folded into
    per-edge gather weights so the device does: gather -> scale&cast ->
    one-hot matmul segment-sum in PSUM -> dense W1/relu/W2 -> AllGather of
    dinv-scaled hidden h2s -> same aggregation machinery -> log_softmax.
  - Aggregation: nodes are bin-packed into "windows" of <=128 nodes whose
    edges occupy 4 fixed-capacity chunk cells (gather src locality for int16
    indices) + 1 self cell.  Segment-sum = sum over 128-edge columns of
    S_col^T-style one-hot matmuls accumulated in PSUM.  The one-hot S is
    built on-device from per-edge dst-rank data via a single is_equal op.
  - All structure (window/cell/column/piece layout) is static and identical
    across cores; per-core variation lives only in input data arrays.

kernel(**inputs) takes FULL inputs and returns the FULL [100000, 64] output.
"""
import os
import numpy as np

os.environ.setdefault("NEURON_SCRATCHPAD_PAGE_SIZE", "256")

import concourse.bacc as bacc
import concourse.tile as tile
import concourse.mybir as mybir
from concourse import bass
from concourse.bass_utils import run_bass_kernel_spmd
from concourse.masks import make_identity

F32 = mybir.dt.float32
BF16 = mybir.dt.bfloat16
I16 = mybir.dt.int16
NPBF16 = mybir.dt.np(BF16)

AF = mybir.ActivationFunctionType
OP = mybir.AluOpType


class Cfg:
    def __init__(self, N, R, F, HID, C, NC, NWIN, NB, K, dense_bf16=False):
        self.N, self.R, self.F, self.HID, self.C, self.NC = N, R, F, HID, C, NC
        self.NWIN, self.NB, self.K = NWIN, NB, K
        self.WN = 128
        assert NWIN % NB == 0
        self.NBATCH = NWIN // NB
        assert (NB * K) % 128 == 0 and K % 16 == 0 and K >= 128
        # HW limit: DMAGatherAnt crashes for num_idxs > 1024 (empirical)
        assert NB * K <= 1024 and NB * 128 <= 1024
        self.NCOL = NB * K // 128          # columns per real call
        self.IDX_REAL = NB * K             # idxs per real call
        self.IDX_SELF = NB * 128           # idxs per self call
        assert N % 4 == 0
        self.CH1 = N // 4                  # L1 chunk rows
        self.CROWS = NWIN * 128            # compact rows per core
        self.GC = NC * self.CROWS          # global compact rows
        assert self.GC % 4 == 0
        self.CH2 = self.GC // 4            # L2 chunk rows
        assert self.CH1 <= 32768 and self.CH2 <= 32768 and self.CROWS <= 32768
        self.dense_bf16 = dense_bf16
        # static column->cell map for real calls: (first cell, crosses boundary)
        self.colmap = []
        for j in range(self.NCOL):
            a = (j * 128) // K
            self.colmap.append((a, (j * 128 + 127) // K != a))
        # stream free-dim widths (per batch)
        self.GIDX_B = 4 * self.IDX_REAL // 16 + self.IDX_SELF // 16
        self.DRA_B = 4 * self.NCOL + NB
        self.DRB_B = 4 * self.NCOL


CFG_FULL = Cfg(N=100000, R=12500, F=128, HID=256, C=64, NC=8,
               NWIN=100, NB=4, K=256)


# ----------------------------------------------------------------------------
# host prep
# ----------------------------------------------------------------------------

def _pack(cnt, cfg):
    """Bin-pack nodes (rows of cnt: per-chunk edge counts) into NWIN windows.
    Returns (assign, rank) or None."""
    NWIN, K, WN = cfg.NWIN, cfg.K, cfg.WN
    degs = cnt.sum(1)
    order = np.argsort(-degs, kind="stable")
    loads = np.zeros((NWIN, 4), np.int64)
    counts = np.zeros(NWIN, np.int64)
    assign = np.full(len(degs), -1, np.int64)
    rank = np.full(len(degs), -1, np.int64)
    for i in order:
        c = cnt[i]
        ok = (counts < WN) & ((loads + c) <= K).all(axis=1)
        if not ok.any():
            return None
        score = (loads + c).max(axis=1) * 1000 + counts
        score = np.where(ok, score, 1 << 40)
        w = int(np.argmin(score))
        assign[i] = w
        rank[i] = counts[w]
        counts[w] += 1
        loads[w] += c
    return assign, rank


def _wrap_idx(a):
    """[..., n] -> [..., 128, n//16]; idx i at [i%16, i//16], replicated x8."""
    lead = a.shape[:-1]
    n = a.shape[-1]
    w = a.reshape(*lead, n // 16, 16)
    w = np.moveaxis(w, -1, -2)
    return np.tile(w, (*([1] * len(lead)), 8, 1)).astype(np.int16)


def _wrap_col(a, dt):
    """[..., n] -> [..., 128, n//128]; slot p at [p%128, p//128]."""
    lead = a.shape[:-1]
    n = a.shape[-1]
    w = a.reshape(*lead, n // 128, 128)
    return np.ascontiguousarray(np.moveaxis(w, -1, -2)).astype(dt)


def _emit_layer(cfg, dl, gsrc, we, self_gidx, self_w, CH, assign, rank):
    """Emit per-core data streams for one layer.

    dl/gsrc/we: real edges (local dst, gather-space src index, weight).
    self_gidx/self_w: per natural-local-node self gather index and weight.
    Returns dict with gidx [128, NBATCH*GIDX_B] i16, dra/drb bf16, gw f32.
    """
    NB, K, NCOL, NBATCH = cfg.NB, cfg.K, cfg.NCOL, cfg.NBATCH
    IDX_REAL, IDX_SELF = cfg.IDX_REAL, cfg.IDX_SELF
    chunk = gsrc // CH
    w = assign[dl]
    r = rank[dl]
    key = w * 4 + chunk
    order = np.argsort(key, kind="stable")
    skey = key[order]
    cellcnt = np.bincount(skey, minlength=cfg.NWIN * 4)
    cellstart = np.concatenate([[0], np.cumsum(cellcnt)[:-1]])
    within = np.arange(len(order)) - cellstart[skey]
    assert within.max(initial=0) < K, f"cell overflow {within.max()} >= {K}"
    bb = w[order] // NB
    wl = w[order] % NB
    slot = wl * K + within
    ch = chunk[order]

    gidx = np.zeros((NBATCH, 4, IDX_REAL), np.int64)
    gwv = np.zeros((NBATCH, 4, IDX_REAL), np.float32)
    rk = np.full((NBATCH, 4, IDX_REAL), -1000.0, np.float32)
    gidx[bb, ch, slot] = gsrc[order] - ch * CH
    gwv[bb, ch, slot] = we[order]
    rk[bb, ch, slot] = r[order]

    slots = np.arange(IDX_REAL)
    cell_of = slots // K
    wlA = ((slots // 128) * 128) // K
    dra = np.where(cell_of[None, None, :] == wlA[None, None, :], rk, -1000.0)
    drb = np.where(cell_of[None, None, :] == (wlA + 1)[None, None, :], rk, -1000.0)
    # (dra/drb/gw emitted as f32 streams; consumed as tensor_scalar scalars)

    # self call
    node_at = np.full((cfg.NWIN, 128), -1, np.int64)
    node_at[assign, rank] = np.arange(len(assign))
    sidx = np.zeros((NBATCH, IDX_SELF), np.int64)
    sw = np.zeros((NBATCH, IDX_SELF), np.float32)
    sdr = np.full((NBATCH, IDX_SELF), -1000.0, np.float32)
    rr = np.arange(IDX_SELF) % 128
    for b in range(NBATCH):
        flat = node_at[b * NB:(b + 1) * NB].reshape(-1)
        valid = flat >= 0
        sidx[b][valid] = self_gidx[flat[valid]]
        sw[b][valid] = self_w[flat[valid]]
        sdr[b][valid] = rr[valid]

    # assemble streams: per batch: [call0..call3, self]
    gidx_parts, dra_parts, drb_parts, gw_parts = [], [], [], []
    for b in range(NBATCH):
        for c in range(4):
            gidx_parts.append(_wrap_idx(gidx[b, c]))
            dra_parts.append(_wrap_col(dra[b, c], np.float32))
            drb_parts.append(_wrap_col(drb[b, c], np.float32))
            gw_parts.append(_wrap_col(gwv[b, c], np.float32))
        gidx_parts.append(_wrap_idx(sidx[b]))
        dra_parts.append(_wrap_col(sdr[b], np.float32))
        gw_parts.append(_wrap_col(sw[b], np.float32))
    return {
        "gidx": np.concatenate(gidx_parts, axis=1),
        "dra": np.concatenate(dra_parts, axis=1),
        "drb": np.concatenate(drb_parts, axis=1),
        "gw": np.concatenate(gw_parts, axis=1),
    }


def prep(x, W1, b1, W2, b2, edge_index, cfg):
    """Full host prep. Returns (in_maps, perm2_list)."""
    N, R, NC = cfg.N, cfg.R, cfg.NC
    src = np.asarray(edge_index[0], np.int64)
    dst = np.asarray(edge_index[1], np.int64)
    x = np.asarray(x, np.float32)
    x_bf = x.astype(NPBF16)
    deg = np.ones(N, np.float32)
    np.add.at(deg, dst, 1.0)
    dinv = (1.0 / np.sqrt(deg)).astype(np.float32)

    owner = dst // R
    per_core = []
    for k in range(NC):
        m = owner == k
        per_core.append((src[m], dst[m] - k * R))

    # L1 packing
    pk1 = []
    for k in range(NC):
        s_k, d_k = per_core[k]
        cnt = np.zeros((R, 4), np.int64)
        np.add.at(cnt, (d_k, s_k // cfg.CH1), 1)
        r = _pack(cnt, cfg)
        assert r is not None, f"L1 packing failed core {k} (K={cfg.K})"
        pk1.append(r)

    cpos = np.empty(N, np.int64)
    for k in range(NC):
        a, rk = pk1[k]
        cpos[k * R:(k + 1) * R] = k * cfg.CROWS + a * 128 + rk

    # L2 packing
    pk2 = []
    for k in range(NC):
        s_k, d_k = per_core[k]
        cnt = np.zeros((R, 4), np.int64)
        np.add.at(cnt, (d_k, cpos[s_k] // cfg.CH2), 1)
        r = _pack(cnt, cfg)
        assert r is not None, f"L2 packing failed core {k} (K={cfg.K})"
        pk2.append(r)

    iota = np.tile(np.arange(128, dtype=np.float32), (128, 1)).astype(NPBF16)
    b1w = np.zeros((128, cfg.HID // 128), np.float32)
    for h in range(cfg.HID):
        b1w[h % 128, h // 128] = b1[h]
    b2bc = np.tile(np.asarray(b2, np.float32), (128, 1))

    in_maps = []
    perm2_list = []
    for k in range(NC):
        s_k, d_k = per_core[k]
        a1, r1 = pk1[k]
        a2, r2 = pk2[k]
        dloc = dinv[k * R:(k + 1) * R]

        st1 = _emit_layer(
            cfg, d_k, s_k, dinv[s_k] * dloc[d_k],
            np.arange(R), dloc * dloc, cfg.CH1, a1, r1)
        st2 = _emit_layer(
            cfg, d_k, cpos[s_k], dloc[d_k],
            a1 * 128 + r1, dloc, cfg.CH2, a2, r2)

        # dinv in L1-compact order (0 on pad rows)
        dc = np.zeros((cfg.NWIN, 128), np.float32)
        dc[a1, r1] = dloc
        perm2_list.append(a2 * 128 + r2)

        in_maps.append({
            "x": x_bf,
            "x_own": np.ascontiguousarray(x_bf[k * R:(k + 1) * R]),
            "W1": np.asarray(W1, np.float32),
            "W2": np.asarray(W2, np.float32),
            "b1w": b1w,
            "b2bc": b2bc,
            "iota": iota,
            "dinv1": np.ascontiguousarray(dc.T),
            "gidx1": st1["gidx"], "dra1": st1["dra"],
            "drb1": st1["drb"], "gw1": st1["gw"],
            "gidx2": st2["gidx"], "dra2": st2["dra"],
            "drb2": st2["drb"], "gw2": st2["gw"],
        })
    return in_maps, perm2_list


# ----------------------------------------------------------------------------
# device program
# ----------------------------------------------------------------------------

def _first_piece_per_bank(cfg, per_bank):
    """PSUM start=True must be issued exactly once per 2KB bank (the HW/sim
    zero-region granularity). Returns bank -> (c, j, lab, wl) of the first
    emitted aggregation matmul targeting that bank. Emission order:
    c in 0..3, j in 0..NCOL-1, A then B; self call last."""
    first = {}
    for c in range(4):
        for j, (a, split) in enumerate(cfg.colmap):
            for wl, lab in ((a, "A"), (a + 1, "B")) if split else ((a, "A"),):
                if wl >= cfg.NB:
                    continue
                bk = wl // per_bank
                if bk not in first:
                    first[bk] = (c, j, lab, wl)
    assert len(first) == (cfg.NB + per_bank - 1) // per_bank
    return first


def build_program(cfg, debug_taps=False):
    nc = bacc.Bacc(None, target_bir_lowering=False, debug=False)
    F, HID, C = cfg.F, cfg.HID, cfg.C
    NB, NCOL, NBATCH = cfg.NB, cfg.NCOL, cfg.NBATCH
    ddt = BF16 if cfg.dense_bf16 else F32

    x = nc.declare_dram_parameter("x", [cfg.N, F], BF16, isOutput=False)
    x_own = nc.declare_dram_parameter("x_own", [cfg.R, F], BF16, isOutput=False)
    W1 = nc.declare_dram_parameter("W1", [F, HID], F32, isOutput=False)
    W2 = nc.declare_dram_parameter("W2", [HID, C], F32, isOutput=False)
    b1w = nc.declare_dram_parameter("b1w", [128, HID // 128], F32, isOutput=False)
    b2bc = nc.declare_dram_parameter("b2bc", [128, C], F32, isOutput=False)
    iota = nc.declare_dram_parameter("iota", [128, 128], BF16, isOutput=False)
    dinv1 = nc.declare_dram_parameter("dinv1", [128, cfg.NWIN], F32, isOutput=False)
    gidx1 = nc.declare_dram_parameter("gidx1", [128, NBATCH * cfg.GIDX_B], I16, isOutput=False)
    dra1 = nc.declare_dram_parameter("dra1", [128, NBATCH * cfg.DRA_B], F32, isOutput=False)
    drb1 = nc.declare_dram_parameter("drb1", [128, NBATCH * cfg.DRB_B], F32, isOutput=False)
    gw1 = nc.declare_dram_parameter("gw1", [128, NBATCH * cfg.DRA_B], F32, isOutput=False)
    gidx2 = nc.declare_dram_parameter("gidx2", [128, NBATCH * cfg.GIDX_B], I16, isOutput=False)
    dra2 = nc.declare_dram_parameter("dra2", [128, NBATCH * cfg.DRA_B], F32, isOutput=False)
    drb2 = nc.declare_dram_parameter("drb2", [128, NBATCH * cfg.DRB_B], F32, isOutput=False)
    gw2 = nc.declare_dram_parameter("gw2", [128, NBATCH * cfg.DRA_B], F32, isOutput=False)
    outc = nc.declare_dram_parameter("outc", [cfg.CROWS, C], F32, isOutput=True)

    # h2s rows padded to F bf16 columns so the L2 gather elem is 256B-aligned;
    # pad columns are never read (matmul slices [:, 0:C]).
    h2s_c = nc.dram_tensor("h2s_c", [cfg.CROWS, F], BF16)
    h2s_full = nc.dram_tensor("h2s_full", [cfg.GC, F], BF16, addr_space="Shared")
    dbg_aggT = None
    if debug_taps:
        dbg_aggT = nc.declare_dram_parameter(
            "dbg_aggT", [128, cfg.CROWS], F32, isOutput=True)

    first_bank = _first_piece_per_bank(cfg, cfg.NB)

    with tile.TileContext(nc) as tc:
        with (
            tc.tile_pool(name="const", bufs=1) as pc,
            tc.tile_pool(name="gpool", bufs=2) as pg,
            tc.tile_pool(name="spool", bufs=2) as ps,
            tc.tile_pool(name="dense", bufs=2) as pd,
            tc.tile_pool(name="psagg", bufs=2, space="PSUM") as ppa,
            tc.tile_pool(name="psdense", bufs=2, space="PSUM") as ppd,
        ):
            # ---- load constants / streams into SBUF
            def load(param, shape, dtype, tag):
                t = pc.tile(shape, dtype, tag=tag)
                nc.sync.dma_start(out=t[:], in_=param[:, :])
                return t

            W1_sb = load(W1, [F, HID], F32, "W1sb")
            W2a_sb = pc.tile([128, C], F32, tag="W2a", name="W2a")
            W2b_sb = pc.tile([128, C], F32, tag="W2b", name="W2b")
            nc.sync.dma_start(out=W2a_sb[:], in_=W2[0:128, :])
            nc.sync.dma_start(out=W2b_sb[:], in_=W2[128:256, :])
            b1_sb = load(b1w, [128, HID // 128], F32, "b1sb")
            b2_sb = load(b2bc, [128, C], F32, "b2sb")
            iota_sb = load(iota, [128, 128], BF16, "iotasb")
            dinv1_sb = load(dinv1, [128, cfg.NWIN], F32, "dinv1sb")
            ident = pc.tile([128, 128], F32, tag="ident", name="ident")
            make_identity(nc, ident[:])

            if cfg.dense_bf16:
                W1_d = pc.tile([F, HID], BF16, tag="W1d", name="W1d")
                W2a_d = pc.tile([128, C], BF16, tag="W2ad", name="W2ad")
                W2b_d = pc.tile([128, C], BF16, tag="W2bd", name="W2bd")
                nc.vector.tensor_copy(W1_d[:], W1_sb[:])
                nc.vector.tensor_copy(W2a_d[:], W2a_sb[:])
                nc.vector.tensor_copy(W2b_d[:], W2b_sb[:])
            else:
                W1_d, W2a_d, W2b_d = W1_sb, W2a_sb, W2b_sb

            streams = {}
            for nm, par, wid, dt in (
                ("gidx1", gidx1, NBATCH * cfg.GIDX_B, I16),
                ("dra1", dra1, NBATCH * cfg.DRA_B, F32),
                ("drb1", drb1, NBATCH * cfg.DRB_B, F32),
                ("gw1", gw1, NBATCH * cfg.DRA_B, F32),
                ("gidx2", gidx2, NBATCH * cfg.GIDX_B, I16),
                ("dra2", dra2, NBATCH * cfg.DRA_B, F32),
                ("drb2", drb2, NBATCH * cfg.DRB_B, F32),
                ("gw2", gw2, NBATCH * cfg.DRA_B, F32),
            ):
                streams[nm] = load(par, [128, wid], dt, "st_" + nm)

            # ---- one layer of aggregation
            def emit_agg_layer(layer, elem, used, chunk_src_ap, self_src_ap, dense_fn):
                """used: number of leading elem-columns that carry real data
                (h2s rows are padded to `elem`; matmuls slice [:, 0:used])."""
                gi = streams[f"gidx{layer}"]
                da = streams[f"dra{layer}"]
                gw = streams[f"gw{layer}"]
                l1 = layer == 1
                nbatch = int(os.environ.get("GCN_NBATCH", str(NBATCH)))
                for b in range(nbatch):
                    go = b * cfg.GIDX_B
                    ao = b * cfg.DRA_B
                    # one psum bank per batch (NB windows side by side)
                    ww = 128 if l1 else C
                    bank = ppa.tile([128, NB * ww], F32, tag="aggps", name="aggps")

                    def wap(wl):
                        return bank[:, wl * ww:(wl + 1) * ww]

                    for c in range(4):
                        if os.environ.get("GCN_NO_GATHER"):
                            continue
                        G = pg.tile([128, NCOL, elem], BF16, tag="G", name="G")
                        nc.gpsimd.dma_gather(
                            G[:], chunk_src_ap(c),
                            gi[:, go + c * (cfg.IDX_REAL // 16):
                               go + (c + 1) * (cfg.IDX_REAL // 16)],
                            cfg.IDX_REAL, cfg.IDX_REAL, elem)
                        if os.environ.get("GCN_NO_S"):
                            continue
                        for j, (wa, split) in enumerate(cfg.colmap):
                            for wl, second in ((wa, False), (wa + 1, True)) if split else ((wa, False),):
                                if wl >= NB:
                                    continue
                                if os.environ.get("GCN_NO_MM"):
                                    continue
                                col = ao + c * NCOL + j
                                # S = onehot(dst_rank) * weight, one DVE op
                                S = ps.tile([128, 128], BF16, tag="S", name="S",
                                            bufs=6)
                                dsrc = streams[f"drb{layer}"] if second else da
                                nc.vector.tensor_scalar(
                                    S[:], iota_sb[:],
                                    dsrc[:, col:col + 1], gw[:, col:col + 1],
                                    OP.is_equal, OP.mult)
                                lab = "B" if second else "A"
                                start = first_bank[0] == (c, j, lab, wl)
                                if l1:
                                    nc.tensor.matmul(
                                        wap(wl), lhsT=G[:, j, 0:used], rhs=S[:],
                                        start=start, stop=False,
                                        skip_group_check=True)
                                else:
                                    nc.tensor.matmul(
                                        wap(wl), lhsT=S[:], rhs=G[:, j, 0:used],
                                        start=start, stop=False,
                                        skip_group_check=True)
                    # self call
                    if os.environ.get("GCN_NO_SELF"):
                        if not os.environ.get("GCN_NO_MM"):
                            dense_fn(b, bank)
                        continue
                    Gs = pg.tile([128, NB, elem], BF16, tag="Gs", name="Gs")
                    so = go + 4 * (cfg.IDX_REAL // 16)
                    nc.gpsimd.dma_gather(
                        Gs[:], self_src_ap,
                        gi[:, so: so + cfg.IDX_SELF // 16],
                        cfg.IDX_SELF, cfg.IDX_SELF, elem)
                    for wl in range(NB):
                        if os.environ.get("GCN_NO_MM"):
                            continue
                        col = ao + 4 * NCOL + wl
                        S = ps.tile([128, 128], BF16, tag="S", name="S", bufs=6)
                        nc.vector.tensor_scalar(
                            S[:], iota_sb[:],
                            da[:, col:col + 1], gw[:, col:col + 1],
                            OP.is_equal, OP.mult)
                        stop = wl == NB - 1
                        if l1:
                            nc.tensor.matmul(
                                wap(wl), lhsT=Gs[:, wl, 0:used], rhs=S[:],
                                start=False, stop=stop, skip_group_check=True)
                        else:
                            nc.tensor.matmul(
                                wap(wl), lhsT=S[:], rhs=Gs[:, wl, 0:used],
                                start=False, stop=stop, skip_group_check=True)
                    if not os.environ.get("GCN_NO_MM"):
                        dense_fn(b, bank)

            # ---- L1 dense tail: aggT [F, crows] -> h2s rows
            def dense_l1(b, bank):
                if os.environ.get("GCN_NO_DENSE"):
                    aggT_sb = pd.tile([128, NB * 128], ddt, tag="aggT", name="aggT")
                    nc.vector.tensor_copy(aggT_sb[:], bank[:])
                    return
                aggT_sb = pd.tile([128, NB * 128], ddt, tag="aggT", name="aggT")
                nc.vector.tensor_copy(aggT_sb[:], bank[:])
                if dbg_aggT is not None:
                    c0 = b * NB * 128
                    nc.sync.dma_start(
                        out=dbg_aggT[:, c0:c0 + NB * 128], in_=aggT_sb[:])
                hstage = pd.tile([128, NB, F], BF16, tag="hst", name="hst")
                nc.vector.memset(hstage[:], 0.0)
                for wl in range(NB):
                    w = b * NB + wl
                    a_sl = aggT_sb[:, wl * 128:(wl + 1) * 128]
                    ph = ppd.tile([128, 256], F32, tag="hT", name="hT")
                    nc.tensor.matmul(ph[:, 0:128], lhsT=W1_d[:, 0:128],
                                     rhs=a_sl, start=True, stop=False,
                                     skip_group_check=True)
                    nc.tensor.matmul(ph[:, 128:256], lhsT=W1_d[:, 128:256],
                                     rhs=a_sl, start=False, stop=True,
                                     skip_group_check=True)
                    hT = pd.tile([128, 256], ddt, tag="hTsb", name="hTsb")
                    nc.scalar.activation(hT[:, 0:128], ph[:, 0:128],
                                         AF.Relu, bias=b1_sb[:, 0:1])
                    nc.scalar.activation(hT[:, 128:256], ph[:, 128:256],
                                         AF.Relu, bias=b1_sb[:, 1:2])
                    p2 = ppd.tile([64, 128], F32, tag="h2T", name="h2T")
                    nc.tensor.matmul(p2[:], lhsT=W2a_d[:], rhs=hT[:, 0:128],
                                     start=True, stop=False,
                                     skip_group_check=True)
                    nc.tensor.matmul(p2[:], lhsT=W2b_d[:], rhs=hT[:, 128:256],
                                     start=False, stop=True,
                                     skip_group_check=True)
                    h2T = pd.tile([64, 128], F32, tag="h2Tsb", name="h2Tsb")
                    nc.vector.tensor_copy(h2T[:], p2[:])
                    pt = ppd.tile([128, 64], F32, tag="tp", name="tp")
                    nc.tensor.transpose(pt[:], h2T[:], ident[0:64, 0:64])
                    nc.vector.tensor_scalar(hstage[:, wl, 0:C], pt[:],
                                            dinv1_sb[:, w:w + 1], None, OP.mult)
                r0 = b * NB * 128
                nc.sync.dma_start(
                    out=h2s_c[r0:r0 + NB * 128, :]
                    .rearrange("(w r) f -> r w f", w=NB),
                    in_=hstage[:])

            def l1_chunk(c):
                return x[c * cfg.CH1:(c + 1) * cfg.CH1, :]


            # ---- L2 dense tail: psum [crows, C] -> +b2 -> exp/accum into
            # persistent staging; one batched Ln + final combine at the end
            # (keeps ACT on a single function per phase: table reloads are
            # ~1.3us each).
            xs_all = pc.tile([128, cfg.NWIN * C], F32, tag="xs_all", name="xs_all")
            nm_all = pc.tile([128, cfg.NWIN], F32, tag="nm_all", name="nm_all")
            ss_all = pc.tile([128, cfg.NWIN], F32, tag="ss_all", name="ss_all")

            def dense_l2(b, bank):
                for wl in range(NB):
                    w = b * NB + wl
                    xs = xs_all[:, w * C:(w + 1) * C]
                    nc.vector.tensor_add(xs, bank[:, wl * C:(wl + 1) * C], b2_sb[:])
                    nm = nm_all[:, w:w + 1]
                    nc.vector.tensor_reduce(nm, xs, mybir.AxisListType.X,
                                            OP.max, negate=True)
                    es = pd.tile([128, C], F32, tag="es", name="es")
                    nc.scalar.activation(es[:], xs, AF.Exp, bias=nm,
                                         accum_out=ss_all[:, w:w + 1])

            def final_l2():
                ls_all = pc.tile([128, cfg.NWIN], F32, tag="ls_all", name="ls_all")
                nc.scalar.activation(ls_all[:], ss_all[:], AF.Ln)
                for b in range(NBATCH):
                    ost = pd.tile([128, NB, C], F32, tag="ost", name="ost")
                    for wl in range(NB):
                        w = b * NB + wl
                        nc.vector.tensor_scalar(
                            ost[:, wl, :], xs_all[:, w * C:(w + 1) * C],
                            nm_all[:, w:w + 1], ls_all[:, w:w + 1],
                            OP.add, OP.subtract)
                    r0 = b * NB * 128
                    nc.sync.dma_start(
                        out=outc[r0:r0 + NB * 128, :]
                        .rearrange("(w r) f -> r w f", w=NB),
                        in_=ost[:])

            def l2_chunk(c):
                return h2s_full[c * cfg.CH2:(c + 1) * cfg.CH2, :]

            phase = int(os.environ.get("GCN_PHASE", "2"))
            repeat = int(os.environ.get("GCN_REPEAT", "1"))

            for _rep in range(repeat):
                emit_agg_layer(1, F, F, l1_chunk, x_own[:, :], dense_l1)

                if phase >= 1:
                    # ---- AllGather h2s
                    nc.gpsimd.collective_compute(
                        "AllGather", OP.bypass,
                        ins=[h2s_c[:, :]],
                        outs=[h2s_full[:, :]],
                        replica_groups=[list(range(cfg.NC))],
                    )
                if phase >= 2:
                    emit_agg_layer(2, F, C, l2_chunk, h2s_c[:, :], dense_l2)
                    final_l2()
            if phase < 2:
                # debug: copy h2s_c straight to outc and stop
                for rr in range(0, cfg.CROWS, 128):
                    tb = pd.tile([128, C], BF16, tag="cpb", name="cpb")
                    nc.sync.dma_start(out=tb[:], in_=h2s_c[rr:rr + 128, 0:C])
                    tt = pd.tile([128, C], F32, tag="cp", name="cp")
                    nc.vector.tensor_copy(tt[:], tb[:])
                    nc.sync.dma_start(out=outc[rr:rr + 128, :], in_=tt[:])


    nc.compile()
    return nc


_PROGRAM_CACHE = {}


def _get_program(cfg):
    key = (cfg.N, cfg.NWIN, cfg.NB, cfg.K, cfg.dense_bf16)
    if key not in _PROGRAM_CACHE:
        _PROGRAM_CACHE[key] = build_program(cfg)
    return _PROGRAM_CACHE[key]


def kernel(x, W1, b1, W2, b2, edge_index):
    cfg = CFG_FULL
    in_maps, perm2 = prep(x, W1, b1, W2, b2, edge_index, cfg)
    nc = _get_program(cfg)
    res = run_bass_kernel_spmd(
        nc, in_maps, core_ids=list(range(cfg.NC)),
        trace=bool(os.environ.get("GCN_TRACE")))
    if res.exec_time_ns is not None:
        print(f"HW exec time: {res.exec_time_ns} ns")
    out = np.empty((cfg.N, cfg.C), np.float32)
    for k in range(cfg.NC):
        out[k * cfg.R:(k + 1) * cfg.R] = res.results[k]["outc"][perm2[k]]
    return out



# revision 55
# speedup vs baseline: 1.5718x; 1.5718x over previous
"""GCN (2-layer, symmetric-normalized A+I) on 8 Trainium2 NeuronCores.

Architecture (v2 — ReduceScatter instead of AllGather):
  - Nodes padded to 102400 and sharded in natural ranges of R=12800 per core.
  - L1 (dst-sharded): core k aggregates x[src] for its own dst rows via
    per-edge DMA gather (4 src chunks x int16 idx) + one-hot matmuls in PSUM,
    then dense W1/relu/W2.  h2 rows (pre-scaled by dinv) stay SBUF-resident.
  - L2 (src-sharded, flipped): core k forms per-edge contribution rows
    w*h2[src] for edges whose src it owns (one-hot build -> PE transpose ->
    matmul vs resident h2 block) and dma_scatter_adds them into 4
    quarter-split f32 partial tensors [25600, 64] (zero-filled host inputs).
  - 4 ReduceScatters (one per quarter) deliver each core the summed agg2 for
    its own nodes; first RS overlaps remaining quarters' compute.
  - Final: + self-loop term (elementwise vs resident h2) + b2 -> log_softmax.

All structure is static and identical across cores (natural node ranges, no
bin packing); per-core variation lives only in input data streams.

kernel(**inputs) takes FULL inputs and returns the FULL [100000, 64] output.
"""
import os
import numpy as np

os.environ.setdefault("NEURON_SCRATCHPAD_PAGE_SIZE", "256")

import concourse.bacc as bacc
import concourse.tile as tile
import concourse.mybir as mybir
from concourse import bass
from concourse.bass_utils import run_bass_kernel_spmd
from concourse.masks import make_identity

F32 = mybir.dt.float32
BF16 = mybir.dt.bfloat16
I16 = mybir.dt.int16
NPBF16 = mybir.dt.np(BF16)

AF = mybir.ActivationFunctionType
OP = mybir.AluOpType


class Cfg:
    def __init__(self):
        self.N = 100000
        self.NPAD = 102400
        self.R = 12800            # nodes per core
        self.F = 128
        self.HID = 256
        self.C = 64
        self.NC = 8
        self.NWIN = 100           # 128-node windows (= L2 src blocks) per core
        self.NB = 4               # windows per L1 batch
        self.NBATCH = 25
        self.K = 256              # L1 cell capacity (window x src-chunk)
        self.CH1 = 25000          # x gather chunk rows (int16 idx range)
        self.NQ = 4               # dst quarters (scatter idx int16 range)
        self.QR = 3200            # quarter rows per core
        self.QTOT = 25600         # quarter tensor rows (all cores)
        self.QCOLS = 200          # L2 columns per quarter (2 per block)
        self.QCALLS = 25          # scatter calls per quarter (8 cols each)
        self.NDUP = 3             # dup-pass scatter calls per quarter
        # L1 stream layout: per batch 32 columns (4 chunks x 8)
        self.L1COLS = 32


CFG_FULL = Cfg()


# ----------------------------------------------------------------------------
# host prep
# ----------------------------------------------------------------------------

def _wrap_idx(a):
    """[n] -> [128, n//16]; idx i at [i%16, i//16], replicated x8."""
    n = a.shape[-1]
    w = a.reshape(n // 16, 16)
    w = np.moveaxis(w, -1, -2)
    return np.tile(w, (8, 1)).astype(np.int16)


def _wrap_col(a, dt):
    """[n] -> [128, n//128]; slot p at [p%128, p//128]."""
    n = a.shape[-1]
    w = a.reshape(n // 128, 128)
    return np.ascontiguousarray(np.moveaxis(w, -1, -2)).astype(dt)


def prep(x, W1, b1, W2, b2, edge_index, cfg):
    N, NPAD, R, NC = cfg.N, cfg.NPAD, cfg.R, cfg.NC
    src = np.asarray(edge_index[0], np.int64)
    dst = np.asarray(edge_index[1], np.int64)
    x = np.asarray(x, np.float32)
    x_bf = x.astype(NPBF16)
    deg = np.ones(N, np.float32)
    np.add.at(deg, dst, 1.0)
    dinv = (1.0 / np.sqrt(deg)).astype(np.float32)
    dinv_pad = np.concatenate([dinv, np.ones(NPAD - N, np.float32)])

    iota128 = np.tile(np.arange(128, dtype=np.float32), (128, 1)).astype(NPBF16)
    b1w = np.zeros((128, cfg.HID // 128), np.float32)
    for h in range(cfg.HID):
        b1w[h % 128, h // 128] = b1[h]
    b2bc = np.tile(np.asarray(b2, np.float32), (128, 1))

    # scatter row permutation within a quarter slab: rr = node % 3200
    #   local = (rr % 128) * 25 + rr // 128  (rank-major so the RS output
    #   loads flat as [128 rank, 25 win, 64])
    def prow_of(d):
        q = (d % R) // cfg.QR
        rr = d % cfg.QR
        local = (rr % 128) * 25 + rr // 128
        return q, (d // R) * cfg.QR + local

    in_maps = []
    for k in range(NC):
        base = k * R
        # ---------------- L1 (dst in own range) ----------------
        m1 = (dst >= base) & (dst < base + R)
        s1, d1 = src[m1], dst[m1] - base
        w1 = d1 // 128           # window 0..99
        r1 = d1 % 128            # rank in window
        c1 = s1 // cfg.CH1       # src chunk 0..3
        # slot within cell, cells capacity K, call = (batch, chunk)
        key = (w1 * 4 + c1)
        order = np.argsort(key, kind="stable")
        skey = key[order]
        cellcnt = np.bincount(skey, minlength=cfg.NWIN * 4)
        assert cellcnt.max() <= cfg.K, f"L1 cell overflow {cellcnt.max()}"
        cellstart = np.concatenate([[0], np.cumsum(cellcnt)[:-1]])
        within = np.arange(len(order)) - cellstart[skey]
        so, wo, co, ro = s1[order], w1[order], c1[order], r1[order]
        b_ = wo // cfg.NB
        wl = wo % cfg.NB
        # slot inside the call: wl*K + within
        slot = wl * cfg.K + within
        NCALL1 = cfg.NBATCH * 4
        gidx1 = np.zeros((NCALL1, 1024), np.int64)
        rk1 = np.full((cfg.NBATCH, cfg.L1COLS, 128), -1000.0, np.float32)
        gwv1 = np.zeros((cfg.NBATCH, cfg.L1COLS, 128), np.float32)
        call = b_ * 4 + co
        gidx1[call, slot] = so - co * cfg.CH1
        colc = co * 8 + slot // 128          # column within batch (0..31)
        rk1[b_, colc, slot % 128] = ro
        gwv1[b_, colc, slot % 128] = dinv[so] * dinv[base + wo * 128 + ro]
        # self-loop folded into the dense tail: x_selfT stream in aggT layout
        # [128 f, (batch, wl, rank)] pre-scaled by dinv^2 (0 on pads)
        nodes = base + np.arange(R)
        valid = nodes < N
        dv2 = np.where(valid, dinv_pad[np.minimum(nodes, N - 1)] ** 2, 0.0)
        xs = np.zeros((R, cfg.F), np.float32)
        xs[valid] = x[nodes[valid]]
        xs *= dv2[:, None]
        # [R, F] -> [F, R] with R ordered (batch, wl, rank) = natural
        x_self = np.ascontiguousarray(xs.T.astype(NPBF16))

        # dinv per (window, rank) for h2 scaling / self-add
        dinv1 = np.where(valid, dinv_pad[np.minimum(nodes, N - 1)], 1.0)
        dinv1 = np.ascontiguousarray(
            dinv1.reshape(cfg.NWIN, 128).T.astype(np.float32))

        # ---------------- L2 (src in own range) ----------------
        m2 = (src >= base) & (src < base + R)
        s2, d2 = src[m2] - base, dst[m2]
        j2 = s2 // 128           # src block
        sr2 = s2 % 128           # src rank in block
        q2, pr2 = prow_of(d2)
        key2 = (q2 * cfg.NWIN + j2)
        order2 = np.argsort(key2, kind="stable")
        skey2 = key2[order2]
        cellcnt2 = np.bincount(skey2, minlength=cfg.NQ * cfg.NWIN)
        assert cellcnt2.max() <= 256, f"L2 cell overflow {cellcnt2.max()}"
        cellstart2 = np.concatenate([[0], np.cumsum(cellcnt2)[:-1]])
        within2 = np.arange(len(order2)) - cellstart2[skey2]
        qo, jo = q2[order2], j2[order2]
        sro, pro = sr2[order2], pr2[order2]
        # per quarter: call g covers blocks 4g..4g+4, slot in call:
        #   ((j%4)*2 + within//128)*128 + within%128
        cslot = ((jo % 4) * 2 + within2 // 128) * 128 + (within2 % 128)
        call2 = qo * cfg.QCALLS + jo // 4
        col2 = qo * cfg.QCOLS + jo * 2 + within2 // 128
        gwo = dinv[dst[m2]][order2]
        # concurrent descriptors of one scatter call race on read-modify-write:
        # a row may appear only ONCE per call.  2nd/3rd hits of a (call, row)
        # go to per-quarter gather-based dup passes (2 levels; max mult is 3).
        ckey = call2 * (cfg.QTOT + 1) + pro
        orderc = np.argsort(ckey, kind="stable")
        n_ = len(orderc)
        same = np.concatenate([[False], ckey[orderc][1:] == ckey[orderc][:-1]])
        ii = np.arange(n_)
        runstart = np.maximum.accumulate(np.where(~same, ii, 0))
        dupl = np.zeros(n_, np.int64)
        dupl[orderc] = ii - runstart
        assert dupl.max() <= 2, f"dup level {dupl.max()}"

        NCALL2 = cfg.NQ * cfg.QCALLS
        gidx2 = np.full((NCALL2, 1024), cfg.QTOT, np.int64)
        rk2 = np.full((cfg.NQ * cfg.QCOLS, 128), -1000.0, np.float32)
        gwv2 = np.zeros((cfg.NQ * cfg.QCOLS, 128), np.float32)
        keep = dupl == 0
        gidx2[call2[keep], cslot[keep]] = pro[keep]
        rk2[col2[keep], cslot[keep] % 128] = sro[keep]
        gwv2[col2[keep], cslot[keep] % 128] = gwo[keep]

        # dup passes: 8 gather/scatter calls (quarter-major, level-minor).
        # Excess edges are assigned per (quarter, row) sequentially so each
        # pass also has unique rows within its call.
        ND = cfg.NDUP
        gidxdup = np.zeros((cfg.NQ * ND, 1024), np.int64)  # h2pad row (src)
        sidxdup = np.full((cfg.NQ * ND, 1024), cfg.QTOT, np.int64)  # pq row
        gwdup = np.zeros((cfg.NQ * ND, 8, 128), np.float32)
        dslot = np.zeros(cfg.NQ * ND, np.int64)
        dm = ~keep
        rowlevel = {}
        for qq, sl, pr_, gw_ in zip(qo[dm], s2[order2][dm], pro[dm], gwo[dm]):
            lv = rowlevel.get((qq, pr_), 0)
            rowlevel[(qq, pr_)] = lv + 1
            assert lv < ND, f"dup pass level overflow row {pr_}"
            dc = qq * ND + lv
            t = dslot[dc]
            assert t < 1024, "dup pass overflow"
            gidxdup[dc, t] = sl
            sidxdup[dc, t] = pr_
            gwdup[dc, t // 128, t % 128] = gw_
            dslot[dc] += 1

        in_maps.append({
            "x": x_bf,
            "W1": np.asarray(W1, np.float32),
            "W2": np.asarray(W2, np.float32),
            "b1w": b1w,
            "b2bc": b2bc,
            "iota": iota128,
            "dinv1": dinv1,
            "x_self": x_self,
            "gidx1": np.concatenate([_wrap_idx(g) for g in gidx1], axis=1),
            "dra1": np.concatenate(
                [_wrap_col(c, np.float32) for c in rk1.reshape(-1, 128)], axis=1),
            "gw1": np.concatenate(
                [_wrap_col(c, np.float32) for c in gwv1.reshape(-1, 128)], axis=1),
            "gidx2": np.concatenate([_wrap_idx(g) for g in gidx2], axis=1),
            "dra2": np.concatenate(
                [_wrap_col(c, np.float32) for c in rk2], axis=1),
            "gw2": np.concatenate(
                [_wrap_col(c, np.float32) for c in gwv2], axis=1),
            "gidxd": np.concatenate([_wrap_idx(g) for g in gidxdup], axis=1),
            "sidxd": np.concatenate(
                [_wrap_idx(g) for g in sidxdup]
                + [_wrap_idx(np.full(128, cfg.QTOT, np.int64))], axis=1),
            "gwd": np.concatenate(
                [_wrap_col(c, np.float32) for c in gwdup.reshape(-1, 128)],
                axis=1),
        })
    return in_maps


# ----------------------------------------------------------------------------
# device program
# ----------------------------------------------------------------------------

def build_program(cfg):
    nc = bacc.Bacc(None, target_bir_lowering=False, debug=False)
    F, HID, C = cfg.F, cfg.HID, cfg.C
    NB, NBATCH, NWIN = cfg.NB, cfg.NBATCH, cfg.NWIN

    x = nc.declare_dram_parameter("x", [cfg.N, F], BF16, isOutput=False)
    W1 = nc.declare_dram_parameter("W1", [F, HID], F32, isOutput=False)
    W2 = nc.declare_dram_parameter("W2", [HID, C], F32, isOutput=False)
    b1w = nc.declare_dram_parameter("b1w", [128, HID // 128], F32, isOutput=False)
    b2bc = nc.declare_dram_parameter("b2bc", [128, C], F32, isOutput=False)
    iota = nc.declare_dram_parameter("iota", [128, 128], BF16, isOutput=False)
    dinv1 = nc.declare_dram_parameter("dinv1", [128, NWIN], F32, isOutput=False)
    x_self = nc.declare_dram_parameter(
        "x_self", [128, NBATCH * NB * 128], BF16, isOutput=False)
    gidx1 = nc.declare_dram_parameter(
        "gidx1", [128, NBATCH * 4 * 64], I16, isOutput=False)
    dra1 = nc.declare_dram_parameter(
        "dra1", [128, NBATCH * cfg.L1COLS], F32, isOutput=False)
    gw1 = nc.declare_dram_parameter(
        "gw1", [128, NBATCH * cfg.L1COLS], F32, isOutput=False)
    gidx2 = nc.declare_dram_parameter(
        "gidx2", [128, cfg.NQ * cfg.QCALLS * 64], I16, isOutput=False)
    dra2 = nc.declare_dram_parameter(
        "dra2", [128, cfg.NQ * cfg.QCOLS], F32, isOutput=False)
    gw2 = nc.declare_dram_parameter(
        "gw2", [128, cfg.NQ * cfg.QCOLS], F32, isOutput=False)
    gidxd = nc.declare_dram_parameter(
        "gidxd", [128, cfg.NQ * cfg.NDUP * 64], I16, isOutput=False)
    sidxd = nc.declare_dram_parameter(
        "sidxd", [128, cfg.NQ * cfg.NDUP * 64 + 8], I16, isOutput=False)
    gwd = nc.declare_dram_parameter(
        "gwd", [128, cfg.NQ * cfg.NDUP * 8], F32, isOutput=False)
    pq = [nc.dram_tensor(f"pq{q}", [cfg.QTOT + 32, C], F32)
          for q in range(cfg.NQ)]
    h2pad = nc.dram_tensor("h2pad", [cfg.R, 128], BF16)
    outc = nc.declare_dram_parameter("outc", [cfg.NQ * 128, 25 * C], F32,
                                     isOutput=True)
    rsq = [nc.dram_tensor(f"rsq{q}", [cfg.QR, C], F32) for q in range(cfg.NQ)]

    with tile.TileContext(nc) as tc:
        with (
            tc.tile_pool(name="const", bufs=1) as pc,
            tc.tile_pool(name="gpool", bufs=2) as pg,
            tc.tile_pool(name="spool", bufs=2) as ps,
            tc.tile_pool(name="dense", bufs=2) as pd,
        ):
            def load(param, shape, dtype, tag):
                t = pc.tile(shape, dtype, tag=tag)
                nc.sync.dma_start(out=t[:], in_=param[:, :])
                return t

            # load order matters: SP serializes its DMAs and holds the engine
            # for the whole transfer, so the streams the L1 pipeline needs
            # first are loaded first.
            gidx1_sb = load(gidx1, [128, NBATCH * 4 * 64], I16, "gidx1sb")
            dra1_sb = load(dra1, [128, NBATCH * cfg.L1COLS], F32, "dra1sb")
            gw1_sb = load(gw1, [128, NBATCH * cfg.L1COLS], F32, "gw1sb")
            iota_sb = load(iota, [128, 128], BF16, "iotasb")
            W1_sb = load(W1, [F, HID], F32, "W1sb")
            xself_sb = pc.tile([128, NBATCH * NB * 128], BF16, tag="xselfsb")
            half = NBATCH * NB * 128 // 2
            nc.sync.dma_start(out=xself_sb[:, 0:half], in_=x_self[:, 0:half])
            W2a_sb = pc.tile([128, C], F32, tag="W2a", name="W2a")
            W2b_sb = pc.tile([128, C], F32, tag="W2b", name="W2b")
            nc.sync.dma_start(out=W2a_sb[:], in_=W2[0:128, :])
            nc.sync.dma_start(out=W2b_sb[:], in_=W2[128:256, :])
            b1_sb = load(b1w, [128, HID // 128], F32, "b1sb")
            b2_sb = load(b2bc, [128, C], F32, "b2sb")
            dinv1_sb = load(dinv1, [128, NWIN], F32, "dinv1sb")
            nc.sync.dma_start(out=xself_sb[:, half:], in_=x_self[:, half:])
            gidx2_sb = load(gidx2, [128, cfg.NQ * cfg.QCALLS * 64], I16, "gidx2sb")
            dra2_sb = load(dra2, [128, cfg.NQ * cfg.QCOLS], F32, "dra2sb")
            gw2_sb = load(gw2, [128, cfg.NQ * cfg.QCOLS], F32, "gw2sb")
            gidxd_sb = load(gidxd, [128, cfg.NQ * cfg.NDUP * 64], I16, "gidxdsb")
            sidxd_sb = load(sidxd, [128, cfg.NQ * cfg.NDUP * 64 + 8], I16,
                            "sidxdsb")
            gwd_sb = load(gwd, [128, cfg.NQ * cfg.NDUP * 8], F32, "gwdsb")
            drainidx_sb = pc.tile([128, 8], I16, tag="drainidx")
            nc.gpsimd.memset(drainidx_sb[:], 0)
            ident = pc.tile([128, 128], F32, tag="ident", name="ident")
            make_identity(nc, ident[:])
            identb = pc.tile([128, 128], BF16, tag="identb", name="identb")
            make_identity(nc, identb[:])

            h2all = pc.tile([128, NWIN * C], BF16, tag="h2all", name="h2all")
            xs_all = pc.tile([128, NWIN * C], F32, tag="xs_all", name="xs_all")
            nm_all = pc.tile([128, NWIN], F32, tag="nm_all", name="nm_all")
            ss_all = pc.tile([128, NWIN], F32, tag="ss_all", name="ss_all")

            # ================= emission helpers =================
            l1_psum = tc.tile_pool(name="psagg", bufs=2, space="PSUM")
            ppa = l1_psum.__enter__()
            l1_psum_d = tc.tile_pool(name="psdense", bufs=2, space="PSUM")
            ppd = l1_psum_d.__enter__()
            l2_psum_t = tc.tile_pool(name="pstr", bufs=2, space="PSUM")
            ppt = l2_psum_t.__enter__()
            l2_psum_c = tc.tile_pool(name="psctr", bufs=2, space="PSUM")
            ppc = l2_psum_c.__enter__()

            def emit_l1_batch(b):
                bank = ppa.tile([128, NB * 128], F32, tag="aggps", name="aggps")
                for c in range(4):
                    G = pg.tile([128, 8, F], BF16, tag="G", name="G", bufs=3)
                    go = (b * 4 + c) * 64
                    nc.gpsimd.dma_gather(
                        G[:], x[c * cfg.CH1:(c + 1) * cfg.CH1, :],
                        gidx1_sb[:, go:go + 64], 1024, 1024, F)
                    for j in range(8):
                        col = b * cfg.L1COLS + c * 8 + j
                        S = ps.tile([128, 128], BF16, tag="S", name="S", bufs=6)
                        nc.vector.tensor_scalar(
                            S[:], iota_sb[:],
                            dra1_sb[:, col:col + 1], gw1_sb[:, col:col + 1],
                            OP.is_equal, OP.mult)
                        wl = j // 2
                        nc.tensor.matmul(
                            bank[:, wl * 128:(wl + 1) * 128],
                            lhsT=G[:, j, :], rhs=S[:],
                            start=(c == 0 and j == 0), stop=(c == 3 and j == 7),
                            skip_group_check=True)
                # dense tail; the self-loop term arrives pre-scaled in aggT
                # layout via the x_self stream and is added during the PSUM
                # evacuation
                aggT_sb = pd.tile([128, NB * 128], F32, tag="aggT", name="aggT")
                nc.vector.tensor_add(
                    aggT_sb[:], bank[:],
                    xself_sb[:, b * NB * 128:(b + 1) * NB * 128])
                for wl in range(NB):
                    w = b * NB + wl
                    a_sl = aggT_sb[:, wl * 128:(wl + 1) * 128]
                    # ph/p2/pt share one 2KB PSUM bank; each start=True
                    # re-zeroes it only after the prior region's reader ran
                    # (relu reads ph before p2's mm; transpose reads the SBUF
                    # copy of p2, so it orders after that copy).
                    dt = ppd.tile([128, 448], F32, tag="dt", name="dt")
                    ph = dt[:, 0:256]
                    nc.tensor.matmul(ph[:, 0:128], lhsT=W1_sb[:, 0:128],
                                     rhs=a_sl, start=True, stop=False,
                                     skip_group_check=True)
                    nc.tensor.matmul(ph[:, 128:256], lhsT=W1_sb[:, 128:256],
                                     rhs=a_sl, start=False, stop=True,
                                     skip_group_check=True)
                    hT = pd.tile([128, 256], F32, tag="hTsb", name="hTsb")
                    nc.scalar.activation(hT[:, 0:128], ph[:, 0:128],
                                         AF.Relu, bias=b1_sb[:, 0:1])
                    nc.scalar.activation(hT[:, 128:256], ph[:, 128:256],
                                         AF.Relu, bias=b1_sb[:, 1:2])
                    p2 = dt[0:64, 256:384]
                    nc.tensor.matmul(p2, lhsT=W2a_sb[:], rhs=hT[:, 0:128],
                                     start=True, stop=False,
                                     skip_group_check=True)
                    nc.tensor.matmul(p2, lhsT=W2b_sb[:], rhs=hT[:, 128:256],
                                     start=False, stop=True,
                                     skip_group_check=True)
                    h2T = pd.tile([64, 128], F32, tag="h2Tsb", name="h2Tsb")
                    nc.scalar.activation(h2T[:], p2, AF.Copy)
                    pt = dt[:, 384:448]
                    nc.tensor.transpose(pt, h2T[:], ident[0:64, 0:64])
                    nc.vector.tensor_scalar(h2all[:, w * C:(w + 1) * C], pt,
                                            dinv1_sb[:, w:w + 1], None, OP.mult)
                # copy of h2 rows in DRAM for the dup-pass gathers (rows padded
                # to 256B; pad columns are never read)
                nc.sync.dma_start(
                    out=h2pad[b * NB * 128:(b + 1) * NB * 128, 0:C]
                    .rearrange("(w r) f -> r w f", w=NB),
                    in_=h2all[:, b * NB * C:(b + 1) * NB * C]
                    .rearrange("p (w f) -> p w f", w=NB))

            def emit_l2_call(q, g):
                ctile = ppc.tile([128, 8 * C], F32, tag="ctr", name="ctr")
                tp8 = ppt.tile([128, 8, 128], BF16, tag="tp8", name="tp8")
                s2 = ps.tile([128, 8, 128], BF16, tag="S2", name="S2", bufs=3)
                for t in range(8):
                    col = q * cfg.QCOLS + g * 8 + t
                    P = ps.tile([128, 128], BF16, tag="P", name="P", bufs=6)
                    nc.vector.tensor_scalar(
                        P[:], iota_sb[:],
                        dra2_sb[:, col:col + 1], gw2_sb[:, col:col + 1],
                        OP.is_equal, OP.mult)
                    nc.tensor.matmul(
                        tp8[:, t, :], lhsT=P[:], rhs=identb[:],
                        is_transpose=True, start=(t == 0), stop=(t == 7),
                        skip_group_check=True)
                nc.vector.tensor_copy(s2[:], tp8[:])
                for cc in range(8):
                    jj = 4 * g + cc // 2
                    nc.tensor.matmul(
                        ctile[:, cc * C:(cc + 1) * C],
                        lhsT=s2[:, cc, :],
                        rhs=h2all[:, jj * C:(jj + 1) * C],
                        start=(cc == 0), stop=(cc == 7),
                        skip_group_check=True)
                sc = pg.tile([128, 8, C], F32, tag="sc", name="sc", bufs=12)
                nc.scalar.activation(sc[:], ctile[:], AF.Copy)
                so = (q * cfg.QCALLS + g) * 64
                return nc.gpsimd.dma_scatter_add(
                    pq[q][:, :], sc[:], gidx2_sb[:, so:so + 64],
                    1024, 1024, C)

            def emit_dup(q, lv):
                dc = q * cfg.NDUP + lv
                Gd = pg.tile([128, 8, 128], BF16, tag="Gd", name="Gd", bufs=2)
                nc.gpsimd.dma_gather(
                    Gd[:], h2pad[:, :],
                    gidxd_sb[:, dc * 64:(dc + 1) * 64], 1024, 1024, 128)
                scd = pg.tile([128, 8, C], F32, tag="scd", name="scd", bufs=2)
                for cc in range(8):
                    nc.vector.tensor_scalar(
                        scd[:, cc, :], Gd[:, cc, 0:C],
                        gwd_sb[:, dc * 8 + cc:dc * 8 + cc + 1], None, OP.mult)
                return nc.gpsimd.dma_scatter_add(
                    pq[q][:, :], scd[:], sidxd_sb[:, dc * 64:(dc + 1) * 64],
                    1024, 1024, C)

            def emit_rs(q):
                # drain gate: scatter->scatter ordering on one tensor is
                # DMA-completion exact (verified), so a dummy zero scatter
                # into pq[q] cannot start until every real scatter drained;
                # the RS RAW-depends on it.
                gidx_gate = sidxd_sb[:, cfg.NQ * cfg.NDUP * 64:]
                dg = nc.gpsimd.dma_scatter_add(
                    pq[q][:, :], zsrc[:, 0:C].rearrange("p (o f) -> p o f", o=1),
                    gidx_gate, 128, 128, C)
                cc = nc.gpsimd.collective_compute(
                    "ReduceScatter", OP.add,
                    ins=[pq[q][0:cfg.QTOT, :]],
                    outs=[rsq[q][:, :]],
                    replica_groups=[list(range(cfg.NC))],
                )
                bass._add_dep_helper(cc.ins, dg.ins, sync=True,
                                     reason="rs-after-scatter-drain")
                return cc

            def emit_final(q):
                rv = pd.tile([128, 25 * C], F32, tag="rv", name="rv")
                nc.sync.dma_start(
                    out=rv[:],
                    in_=rsq[q][:, :].rearrange("(p i) f -> p (i f)", p=128))
                for i in range(25):
                    w = q * 25 + i
                    xsl = xs_all[:, w * C:(w + 1) * C]
                    nc.vector.tensor_scalar(
                        xsl, h2all[:, w * C:(w + 1) * C],
                        dinv1_sb[:, w:w + 1], None, OP.mult)
                    nc.vector.tensor_add(xsl, xsl, rv[:, i * C:(i + 1) * C])
                    nc.vector.tensor_add(xsl, xsl, b2_sb[:])
                    nm = nm_all[:, w:w + 1]
                    nc.vector.tensor_reduce(nm, xsl, mybir.AxisListType.X,
                                            OP.max, negate=True)
                    es = pd.tile([128, C], F32, tag="es", name="es")
                    nc.scalar.activation(es[:], xsl, AF.Exp, bias=nm,
                                         accum_out=ss_all[:, w:w + 1])
                ls = pc.tile([128, 25], F32, tag=f"ls{q}", name=f"ls{q}")
                nc.scalar.activation(ls[:], ss_all[:, q * 25:(q + 1) * 25],
                                     AF.Ln)
                ost = pd.tile([128, 25 * C], F32, tag="ost", name="ost")
                for i in range(25):
                    w = q * 25 + i
                    nc.vector.tensor_scalar(
                        ost[:, i * C:(i + 1) * C], xs_all[:, w * C:(w + 1) * C],
                        nm_all[:, w:w + 1], ls[:, i:i + 1],
                        OP.add, OP.subtract)
                nc.sync.dma_start(
                    out=outc[q * 128:(q + 1) * 128, :], in_=ost[:])

            # ============== interleaved schedule ==============
            # L1 batch b + quarter-0/1 scatter calls (delayed by DLY batches so
            # the on-device zeroing of pq finishes before the first scatter hits
            # the in-order Pool queue); quarters 2/3 after L1 (overlap RS 0/1);
            # finals pipeline after RS.
            DLY = 8
            # zero the partial tensors up front; SP-issued DMAs serialize with
            # each other but run in parallel with Pool's SWDGE transfers
            zsrc = pc.tile([128, 25 * C], F32, tag="zsrc", name="zsrc")
            nc.vector.memset(zsrc[:], 0.0)
            for q in range(cfg.NQ):
                for g2 in range(8):
                    nc.sync.dma_start(
                        out=pq[q][3200 * g2:3200 * (g2 + 1), :]
                        .rearrange("(p i) f -> p (i f)", p=128),
                        in_=zsrc[:])
            for b in range(NBATCH):
                emit_l1_batch(b)
                if b >= DLY:
                    emit_l2_call(0, b - DLY)
                    emit_l2_call(1, b - DLY)
            for g in range(cfg.QCALLS - DLY, cfg.QCALLS):
                emit_l2_call(0, g)
                emit_l2_call(1, g)
            for lv in range(cfg.NDUP):
                emit_dup(0, lv)
            for lv in range(cfg.NDUP):
                emit_dup(1, lv)
            # alternating tail: RS_q (on the collective cores) overlaps the
            # NEXT quarter's compute; its scatters drain between collectives
            # (a running collective blocks SWDGE DMA).
            emit_rs(0)
            emit_rs(1)
            for g in range(cfg.QCALLS):
                emit_l2_call(2, g)
            for lv in range(cfg.NDUP):
                emit_dup(2, lv)
            for g in range(cfg.QCALLS):
                emit_l2_call(3, g)
            for lv in range(cfg.NDUP):
                emit_dup(3, lv)
            emit_final(0)
            emit_final(1)
            emit_rs(2)
            emit_rs(3)
            emit_final(2)
            emit_final(3)
            l2_psum_c.__exit__(None, None, None)
            l2_psum_t.__exit__(None, None, None)
            l1_psum_d.__exit__(None, None, None)
            l1_psum.__exit__(None, None, None)

    nc.compile()
    return nc


_PROGRAM_CACHE = {}


def _get_program(cfg):
    if "prog" not in _PROGRAM_CACHE:
        _PROGRAM_CACHE["prog"] = build_program(cfg)
    return _PROGRAM_CACHE["prog"]


def kernel(x, W1, b1, W2, b2, edge_index):
    cfg = CFG_FULL
    in_maps = prep(x, W1, b1, W2, b2, edge_index, cfg)
    nc = _get_program(cfg)
    res = run_bass_kernel_spmd(
        nc, in_maps, core_ids=list(range(cfg.NC)),
        trace=bool(os.environ.get("GCN_TRACE")))
    if res.exec_time_ns is not None:
        print(f"HW exec time: {res.exec_time_ns} ns")
    out = np.empty((cfg.N, cfg.C), np.float32)
    for k in range(cfg.NC):
        arr = res.results[k]["outc"].reshape(cfg.NQ, 128, 25, cfg.C)
        # node (q, p, i) = k*R + q*3200 + i*128 + p
        blk = arr.transpose(0, 2, 1, 3).reshape(cfg.NQ * 25 * 128, cfg.C)
        n0 = k * cfg.R
        n1 = min(cfg.N, n0 + cfg.R)
        out[n0:n1] = blk[:n1 - n0]
    return out


# revision 56
# speedup vs baseline: 1.5814x; 1.0061x over previous
"""GCN (2-layer, symmetric-normalized A+I) on 8 Trainium2 NeuronCores.

Architecture (v2 — ReduceScatter instead of AllGather):
  - Nodes padded to 102400 and sharded in natural ranges of R=12800 per core.
  - L1 (dst-sharded): core k aggregates x[src] for its own dst rows via
    per-edge DMA gather (4 src chunks x int16 idx) + one-hot matmuls in PSUM,
    then dense W1/relu/W2.  h2 rows (pre-scaled by dinv) stay SBUF-resident.
  - L2 (src-sharded, flipped): core k forms per-edge contribution rows
    w*h2[src] for edges whose src it owns (one-hot build -> PE transpose ->
    matmul vs resident h2 block) and dma_scatter_adds them into 4
    quarter-split f32 partial tensors [25600, 64] (zero-filled host inputs).
  - 4 ReduceScatters (one per quarter) deliver each core the summed agg2 for
    its own nodes; first RS overlaps remaining quarters' compute.
  - Final: + self-loop term (elementwise vs resident h2) + b2 -> log_softmax.

All structure is static and identical across cores (natural node ranges, no
bin packing); per-core variation lives only in input data streams.

kernel(**inputs) takes FULL inputs and returns the FULL [100000, 64] output.
"""
import os
import numpy as np

os.environ.setdefault("NEURON_SCRATCHPAD_PAGE_SIZE", "256")

import concourse.bacc as bacc
import concourse.tile as tile
import concourse.mybir as mybir
from concourse import bass
from concourse.bass_utils import run_bass_kernel_spmd
from concourse.masks import make_identity

F32 = mybir.dt.float32
BF16 = mybir.dt.bfloat16
I16 = mybir.dt.int16
NPBF16 = mybir.dt.np(BF16)

AF = mybir.ActivationFunctionType
OP = mybir.AluOpType


class Cfg:
    def __init__(self):
        self.N = 100000
        self.NPAD = 102400
        self.R = 12800            # nodes per core
        self.F = 128
        self.HID = 256
        self.C = 64
        self.NC = 8
        self.NWIN = 100           # 128-node windows (= L2 src blocks) per core
        self.NB = 4               # windows per L1 batch
        self.NBATCH = 25
        self.K = 256              # L1 cell capacity (window x src-chunk)
        self.CH1 = 25000          # x gather chunk rows (int16 idx range)
        self.NQ = 4               # dst quarters (scatter idx int16 range)
        self.QR = 3200            # quarter rows per core
        self.QTOT = 25600         # quarter tensor rows (all cores)
        self.QCOLS = 200          # L2 columns per quarter (2 per block)
        self.QCALLS = 25          # scatter calls per quarter (8 cols each)
        self.NDUP = 3             # dup-pass scatter calls per quarter
        # L1 stream layout: per batch 32 columns (4 chunks x 8)
        self.L1COLS = 32


CFG_FULL = Cfg()


# ----------------------------------------------------------------------------
# host prep
# ----------------------------------------------------------------------------

def _wrap_idx(a):
    """[n] -> [128, n//16]; idx i at [i%16, i//16], replicated x8."""
    n = a.shape[-1]
    w = a.reshape(n // 16, 16)
    w = np.moveaxis(w, -1, -2)
    return np.tile(w, (8, 1)).astype(np.int16)


def _wrap_col(a, dt):
    """[n] -> [128, n//128]; slot p at [p%128, p//128]."""
    n = a.shape[-1]
    w = a.reshape(n // 128, 128)
    return np.ascontiguousarray(np.moveaxis(w, -1, -2)).astype(dt)


def prep(x, W1, b1, W2, b2, edge_index, cfg):
    N, NPAD, R, NC = cfg.N, cfg.NPAD, cfg.R, cfg.NC
    src = np.asarray(edge_index[0], np.int64)
    dst = np.asarray(edge_index[1], np.int64)
    x = np.asarray(x, np.float32)
    x_bf = x.astype(NPBF16)
    deg = np.ones(N, np.float32)
    np.add.at(deg, dst, 1.0)
    dinv = (1.0 / np.sqrt(deg)).astype(np.float32)
    dinv_pad = np.concatenate([dinv, np.ones(NPAD - N, np.float32)])

    iota128 = np.tile(np.arange(128, dtype=np.float32), (128, 1)).astype(NPBF16)
    b1w = np.zeros((128, cfg.HID // 128), np.float32)
    for h in range(cfg.HID):
        b1w[h % 128, h // 128] = b1[h]
    b2bc = np.tile(np.asarray(b2, np.float32), (128, 1))

    # scatter row permutation within a quarter slab: rr = node % 3200
    #   local = (rr % 128) * 25 + rr // 128  (rank-major so the RS output
    #   loads flat as [128 rank, 25 win, 64])
    def prow_of(d):
        q = (d % R) // cfg.QR
        rr = d % cfg.QR
        local = (rr % 128) * 25 + rr // 128
        return q, (d // R) * cfg.QR + local

    in_maps = []
    for k in range(NC):
        base = k * R
        # ---------------- L1 (dst in own range) ----------------
        m1 = (dst >= base) & (dst < base + R)
        s1, d1 = src[m1], dst[m1] - base
        w1 = d1 // 128           # window 0..99
        r1 = d1 % 128            # rank in window
        c1 = s1 // cfg.CH1       # src chunk 0..3
        # slot within cell, cells capacity K, call = (batch, chunk)
        key = (w1 * 4 + c1)
        order = np.argsort(key, kind="stable")
        skey = key[order]
        cellcnt = np.bincount(skey, minlength=cfg.NWIN * 4)
        assert cellcnt.max() <= cfg.K, f"L1 cell overflow {cellcnt.max()}"
        cellstart = np.concatenate([[0], np.cumsum(cellcnt)[:-1]])
        within = np.arange(len(order)) - cellstart[skey]
        so, wo, co, ro = s1[order], w1[order], c1[order], r1[order]
        b_ = wo // cfg.NB
        wl = wo % cfg.NB
        # slot inside the call: wl*K + within
        slot = wl * cfg.K + within
        NCALL1 = cfg.NBATCH * 4
        gidx1 = np.zeros((NCALL1, 1024), np.int64)
        rk1 = np.full((cfg.NBATCH, cfg.L1COLS, 128), -1000.0, np.float32)
        gwv1 = np.zeros((cfg.NBATCH, cfg.L1COLS, 128), np.float32)
        call = b_ * 4 + co
        gidx1[call, slot] = so - co * cfg.CH1
        colc = co * 8 + slot // 128          # column within batch (0..31)
        rk1[b_, colc, slot % 128] = ro
        gwv1[b_, colc, slot % 128] = dinv[so] * dinv[base + wo * 128 + ro]
        # self-loop folded into the dense tail: x_selfT stream in aggT layout
        # [128 f, (batch, wl, rank)] pre-scaled by dinv^2 (0 on pads)
        nodes = base + np.arange(R)
        valid = nodes < N
        dv2 = np.where(valid, dinv_pad[np.minimum(nodes, N - 1)] ** 2, 0.0)
        xs = np.zeros((R, cfg.F), np.float32)
        xs[valid] = x[nodes[valid]]
        xs *= dv2[:, None]
        # [R, F] -> [F, R] with R ordered (batch, wl, rank) = natural
        x_self = np.ascontiguousarray(xs.T.astype(NPBF16))

        # dinv per (window, rank) for h2 scaling / self-add
        dinv1 = np.where(valid, dinv_pad[np.minimum(nodes, N - 1)], 1.0)
        dinv1 = np.ascontiguousarray(
            dinv1.reshape(cfg.NWIN, 128).T.astype(np.float32))

        # ---------------- L2 (src in own range) ----------------
        m2 = (src >= base) & (src < base + R)
        s2, d2 = src[m2] - base, dst[m2]
        j2 = s2 // 128           # src block
        sr2 = s2 % 128           # src rank in block
        q2, pr2 = prow_of(d2)
        key2 = (q2 * cfg.NWIN + j2)
        order2 = np.argsort(key2, kind="stable")
        skey2 = key2[order2]
        cellcnt2 = np.bincount(skey2, minlength=cfg.NQ * cfg.NWIN)
        assert cellcnt2.max() <= 256, f"L2 cell overflow {cellcnt2.max()}"
        cellstart2 = np.concatenate([[0], np.cumsum(cellcnt2)[:-1]])
        within2 = np.arange(len(order2)) - cellstart2[skey2]
        qo, jo = q2[order2], j2[order2]
        sro, pro = sr2[order2], pr2[order2]
        # per quarter: call g covers blocks 4g..4g+4, slot in call:
        #   ((j%4)*2 + within//128)*128 + within%128
        cslot = ((jo % 4) * 2 + within2 // 128) * 128 + (within2 % 128)
        call2 = qo * cfg.QCALLS + jo // 4
        col2 = qo * cfg.QCOLS + jo * 2 + within2 // 128
        gwo = dinv[dst[m2]][order2]
        # concurrent descriptors of one scatter call race on read-modify-write:
        # a row may appear only ONCE per call.  2nd/3rd hits of a (call, row)
        # go to per-quarter gather-based dup passes (2 levels; max mult is 3).
        ckey = call2 * (cfg.QTOT + 1) + pro
        orderc = np.argsort(ckey, kind="stable")
        n_ = len(orderc)
        same = np.concatenate([[False], ckey[orderc][1:] == ckey[orderc][:-1]])
        ii = np.arange(n_)
        runstart = np.maximum.accumulate(np.where(~same, ii, 0))
        dupl = np.zeros(n_, np.int64)
        dupl[orderc] = ii - runstart
        assert dupl.max() <= 2, f"dup level {dupl.max()}"

        NCALL2 = cfg.NQ * cfg.QCALLS
        gidx2 = np.full((NCALL2, 1024), cfg.QTOT, np.int64)
        rk2 = np.full((cfg.NQ * cfg.QCOLS, 128), -1000.0, np.float32)
        gwv2 = np.zeros((cfg.NQ * cfg.QCOLS, 128), np.float32)
        keep = dupl == 0
        gidx2[call2[keep], cslot[keep]] = pro[keep]
        rk2[col2[keep], cslot[keep] % 128] = sro[keep]
        gwv2[col2[keep], cslot[keep] % 128] = gwo[keep]

        # dup passes: 8 gather/scatter calls (quarter-major, level-minor).
        # Excess edges are assigned per (quarter, row) sequentially so each
        # pass also has unique rows within its call.
        ND = cfg.NDUP
        gidxdup = np.zeros((cfg.NQ * ND, 1024), np.int64)  # h2pad row (src)
        sidxdup = np.full((cfg.NQ * ND, 1024), cfg.QTOT, np.int64)  # pq row
        gwdup = np.zeros((cfg.NQ * ND, 8, 128), np.float32)
        dslot = np.zeros(cfg.NQ * ND, np.int64)
        dm = ~keep
        rowlevel = {}
        for qq, sl, pr_, gw_ in zip(qo[dm], s2[order2][dm], pro[dm], gwo[dm]):
            lv = rowlevel.get((qq, pr_), 0)
            rowlevel[(qq, pr_)] = lv + 1
            assert lv < ND, f"dup pass level overflow row {pr_}"
            dc = qq * ND + lv
            t = dslot[dc]
            assert t < 1024, "dup pass overflow"
            gidxdup[dc, t] = sl
            sidxdup[dc, t] = pr_
            gwdup[dc, t // 128, t % 128] = gw_
            dslot[dc] += 1

        in_maps.append({
            "x": x_bf,
            "W1": np.asarray(W1, np.float32),
            "W2": np.asarray(W2, np.float32),
            "b1w": b1w,
            "b2bc": b2bc,
            "iota": iota128,
            "dinv1": dinv1,
            "x_self": x_self,
            "gidx1": np.concatenate([_wrap_idx(g) for g in gidx1], axis=1),
            "dra1": np.concatenate(
                [_wrap_col(c, np.float32) for c in rk1.reshape(-1, 128)], axis=1),
            "gw1": np.concatenate(
                [_wrap_col(c, np.float32) for c in gwv1.reshape(-1, 128)], axis=1),
            "gidx2": np.concatenate([_wrap_idx(g) for g in gidx2], axis=1),
            "dra2": np.concatenate(
                [_wrap_col(c, np.float32) for c in rk2], axis=1),
            "gw2": np.concatenate(
                [_wrap_col(c, np.float32) for c in gwv2], axis=1),
            "gidxd": np.concatenate([_wrap_idx(g) for g in gidxdup], axis=1),
            "sidxd": np.concatenate(
                [_wrap_idx(g) for g in sidxdup]
                + [_wrap_idx(np.full(128, cfg.QTOT, np.int64))], axis=1),
            "gwd": np.concatenate(
                [_wrap_col(c, np.float32) for c in gwdup.reshape(-1, 128)],
                axis=1),
        })
    return in_maps


# ----------------------------------------------------------------------------
# device program
# ----------------------------------------------------------------------------

def build_program(cfg):
    nc = bacc.Bacc(None, target_bir_lowering=False, debug=False)
    F, HID, C = cfg.F, cfg.HID, cfg.C
    NB, NBATCH, NWIN = cfg.NB, cfg.NBATCH, cfg.NWIN

    x = nc.declare_dram_parameter("x", [cfg.N, F], BF16, isOutput=False)
    W1 = nc.declare_dram_parameter("W1", [F, HID], F32, isOutput=False)
    W2 = nc.declare_dram_parameter("W2", [HID, C], F32, isOutput=False)
    b1w = nc.declare_dram_parameter("b1w", [128, HID // 128], F32, isOutput=False)
    b2bc = nc.declare_dram_parameter("b2bc", [128, C], F32, isOutput=False)
    iota = nc.declare_dram_parameter("iota", [128, 128], BF16, isOutput=False)
    dinv1 = nc.declare_dram_parameter("dinv1", [128, NWIN], F32, isOutput=False)
    x_self = nc.declare_dram_parameter(
        "x_self", [128, NBATCH * NB * 128], BF16, isOutput=False)
    gidx1 = nc.declare_dram_parameter(
        "gidx1", [128, NBATCH * 4 * 64], I16, isOutput=False)
    dra1 = nc.declare_dram_parameter(
        "dra1", [128, NBATCH * cfg.L1COLS], F32, isOutput=False)
    gw1 = nc.declare_dram_parameter(
        "gw1", [128, NBATCH * cfg.L1COLS], F32, isOutput=False)
    gidx2 = nc.declare_dram_parameter(
        "gidx2", [128, cfg.NQ * cfg.QCALLS * 64], I16, isOutput=False)
    dra2 = nc.declare_dram_parameter(
        "dra2", [128, cfg.NQ * cfg.QCOLS], F32, isOutput=False)
    gw2 = nc.declare_dram_parameter(
        "gw2", [128, cfg.NQ * cfg.QCOLS], F32, isOutput=False)
    gidxd = nc.declare_dram_parameter(
        "gidxd", [128, cfg.NQ * cfg.NDUP * 64], I16, isOutput=False)
    sidxd = nc.declare_dram_parameter(
        "sidxd", [128, cfg.NQ * cfg.NDUP * 64 + 8], I16, isOutput=False)
    gwd = nc.declare_dram_parameter(
        "gwd", [128, cfg.NQ * cfg.NDUP * 8], F32, isOutput=False)
    pq = [nc.dram_tensor(f"pq{q}", [cfg.QTOT + 32, C], F32)
          for q in range(cfg.NQ)]
    h2pad = nc.dram_tensor("h2pad", [cfg.R, 128], BF16)
    outc = nc.declare_dram_parameter("outc", [cfg.NQ * 128, 25 * C], F32,
                                     isOutput=True)
    rsq = [nc.dram_tensor(f"rsq{q}", [cfg.QR, C], F32) for q in range(cfg.NQ)]

    with tile.TileContext(nc) as tc:
        with (
            tc.tile_pool(name="const", bufs=1) as pc,
            tc.tile_pool(name="gpool", bufs=2) as pg,
            tc.tile_pool(name="spool", bufs=2) as ps,
            tc.tile_pool(name="dense", bufs=2) as pd,
        ):
            def load(param, shape, dtype, tag):
                t = pc.tile(shape, dtype, tag=tag)
                nc.sync.dma_start(out=t[:], in_=param[:, :])
                return t

            # load order matters: SP serializes its DMAs and holds the engine
            # for the whole transfer, so the streams the L1 pipeline needs
            # first are loaded first.
            gidx1_sb = load(gidx1, [128, NBATCH * 4 * 64], I16, "gidx1sb")
            dra1_sb = load(dra1, [128, NBATCH * cfg.L1COLS], F32, "dra1sb")
            gw1_sb = load(gw1, [128, NBATCH * cfg.L1COLS], F32, "gw1sb")
            iota_sb = load(iota, [128, 128], BF16, "iotasb")
            W1_sb = load(W1, [F, HID], F32, "W1sb")
            xself_sb = pc.tile([128, NBATCH * NB * 128], BF16, tag="xselfsb")
            half = NBATCH * NB * 128 // 2
            nc.sync.dma_start(out=xself_sb[:, 0:half], in_=x_self[:, 0:half])
            W2a_sb = pc.tile([128, C], F32, tag="W2a", name="W2a")
            W2b_sb = pc.tile([128, C], F32, tag="W2b", name="W2b")
            nc.sync.dma_start(out=W2a_sb[:], in_=W2[0:128, :])
            nc.sync.dma_start(out=W2b_sb[:], in_=W2[128:256, :])
            b1_sb = load(b1w, [128, HID // 128], F32, "b1sb")
            b2_sb = load(b2bc, [128, C], F32, "b2sb")
            dinv1_sb = load(dinv1, [128, NWIN], F32, "dinv1sb")
            nc.sync.dma_start(out=xself_sb[:, half:], in_=x_self[:, half:])
            gidx2_sb = load(gidx2, [128, cfg.NQ * cfg.QCALLS * 64], I16, "gidx2sb")
            dra2_sb = load(dra2, [128, cfg.NQ * cfg.QCOLS], F32, "dra2sb")
            gw2_sb = load(gw2, [128, cfg.NQ * cfg.QCOLS], F32, "gw2sb")
            gidxd_sb = load(gidxd, [128, cfg.NQ * cfg.NDUP * 64], I16, "gidxdsb")
            sidxd_sb = load(sidxd, [128, cfg.NQ * cfg.NDUP * 64 + 8], I16,
                            "sidxdsb")
            gwd_sb = load(gwd, [128, cfg.NQ * cfg.NDUP * 8], F32, "gwdsb")
            drainidx_sb = pc.tile([128, 8], I16, tag="drainidx")
            nc.gpsimd.memset(drainidx_sb[:], 0)
            ident = pc.tile([128, 128], F32, tag="ident", name="ident")
            make_identity(nc, ident[:])
            identb = pc.tile([128, 128], BF16, tag="identb", name="identb")
            make_identity(nc, identb[:])

            h2all = pc.tile([128, NWIN * C], BF16, tag="h2all", name="h2all")
            xs_all = pc.tile([128, NWIN * C], F32, tag="xs_all", name="xs_all")
            nm_all = pc.tile([128, NWIN], F32, tag="nm_all", name="nm_all")
            ss_all = pc.tile([128, NWIN], F32, tag="ss_all", name="ss_all")

            # ================= emission helpers =================
            l1_psum = tc.tile_pool(name="psagg", bufs=2, space="PSUM")
            ppa = l1_psum.__enter__()
            l1_psum_d = tc.tile_pool(name="psdense", bufs=2, space="PSUM")
            ppd = l1_psum_d.__enter__()
            l2_psum_t = tc.tile_pool(name="pstr", bufs=2, space="PSUM")
            ppt = l2_psum_t.__enter__()
            l2_psum_c = tc.tile_pool(name="psctr", bufs=2, space="PSUM")
            ppc = l2_psum_c.__enter__()

            def emit_l1_batch(b):
                bank = ppa.tile([128, NB * 128], F32, tag="aggps", name="aggps")
                for c in range(4):
                    G = pg.tile([128, 8, F], BF16, tag="G", name="G", bufs=3)
                    go = (b * 4 + c) * 64
                    nc.gpsimd.dma_gather(
                        G[:], x[c * cfg.CH1:(c + 1) * cfg.CH1, :],
                        gidx1_sb[:, go:go + 64], 1024, 1024, F)
                    for j in range(8):
                        col = b * cfg.L1COLS + c * 8 + j
                        S = ps.tile([128, 128], BF16, tag="S", name="S", bufs=6)
                        nc.vector.tensor_scalar(
                            S[:], iota_sb[:],
                            dra1_sb[:, col:col + 1], gw1_sb[:, col:col + 1],
                            OP.is_equal, OP.mult)
                        wl = j // 2
                        nc.tensor.matmul(
                            bank[:, wl * 128:(wl + 1) * 128],
                            lhsT=G[:, j, :], rhs=S[:],
                            start=(c == 0 and j == 0), stop=(c == 3 and j == 7),
                            skip_group_check=True)
                # dense tail; the self-loop term arrives pre-scaled in aggT
                # layout via the x_self stream and is added during the PSUM
                # evacuation
                aggT_sb = pd.tile([128, NB * 128], F32, tag="aggT", name="aggT")
                nc.vector.tensor_add(
                    aggT_sb[:], bank[:],
                    xself_sb[:, b * NB * 128:(b + 1) * NB * 128])
                for wl in range(NB):
                    w = b * NB + wl
                    a_sl = aggT_sb[:, wl * 128:(wl + 1) * 128]
                    # ph/p2/pt share one 2KB PSUM bank; each start=True
                    # re-zeroes it only after the prior region's reader ran
                    # (relu reads ph before p2's mm; transpose reads the SBUF
                    # copy of p2, so it orders after that copy).
                    dt = ppd.tile([128, 448], F32, tag="dt", name="dt")
                    ph = dt[:, 0:256]
                    nc.tensor.matmul(ph[:, 0:128], lhsT=W1_sb[:, 0:128],
                                     rhs=a_sl, start=True, stop=False,
                                     skip_group_check=True)
                    nc.tensor.matmul(ph[:, 128:256], lhsT=W1_sb[:, 128:256],
                                     rhs=a_sl, start=False, stop=True,
                                     skip_group_check=True)
                    hT = pd.tile([128, 256], F32, tag="hTsb", name="hTsb")
                    nc.scalar.activation(hT[:, 0:128], ph[:, 0:128],
                                         AF.Relu, bias=b1_sb[:, 0:1])
                    nc.scalar.activation(hT[:, 128:256], ph[:, 128:256],
                                         AF.Relu, bias=b1_sb[:, 1:2])
                    p2 = dt[0:64, 256:384]
                    nc.tensor.matmul(p2, lhsT=W2a_sb[:], rhs=hT[:, 0:128],
                                     start=True, stop=False,
                                     skip_group_check=True)
                    nc.tensor.matmul(p2, lhsT=W2b_sb[:], rhs=hT[:, 128:256],
                                     start=False, stop=True,
                                     skip_group_check=True)
                    h2T = pd.tile([64, 128], F32, tag="h2Tsb", name="h2Tsb")
                    nc.scalar.activation(h2T[:], p2, AF.Copy)
                    pt = dt[:, 384:448]
                    nc.tensor.transpose(pt, h2T[:], ident[0:64, 0:64])
                    nc.vector.tensor_scalar(h2all[:, w * C:(w + 1) * C], pt,
                                            dinv1_sb[:, w:w + 1], None, OP.mult)
                # copy of h2 rows in DRAM for the dup-pass gathers (rows padded
                # to 256B; pad columns are never read)
                nc.sync.dma_start(
                    out=h2pad[b * NB * 128:(b + 1) * NB * 128, 0:C]
                    .rearrange("(w r) f -> r w f", w=NB),
                    in_=h2all[:, b * NB * C:(b + 1) * NB * C]
                    .rearrange("p (w f) -> p w f", w=NB))

            def emit_l2_call(q, g):
                ctile = ppc.tile([128, 8 * C], F32, tag="ctr", name="ctr")
                tp8 = ppt.tile([128, 8, 128], BF16, tag="tp8", name="tp8")
                s2 = ps.tile([128, 8, 128], BF16, tag="S2", name="S2", bufs=3)
                for t in range(8):
                    col = q * cfg.QCOLS + g * 8 + t
                    P = ps.tile([128, 128], BF16, tag="P", name="P", bufs=6)
                    nc.vector.tensor_scalar(
                        P[:], iota_sb[:],
                        dra2_sb[:, col:col + 1], gw2_sb[:, col:col + 1],
                        OP.is_equal, OP.mult)
                    nc.tensor.matmul(
                        tp8[:, t, :], lhsT=P[:], rhs=identb[:],
                        is_transpose=True, start=(t == 0), stop=(t == 7),
                        skip_group_check=True)
                nc.vector.tensor_copy(s2[:], tp8[:])
                for cc in range(8):
                    jj = 4 * g + cc // 2
                    nc.tensor.matmul(
                        ctile[:, cc * C:(cc + 1) * C],
                        lhsT=s2[:, cc, :],
                        rhs=h2all[:, jj * C:(jj + 1) * C],
                        start=(cc == 0), stop=(cc == 7),
                        skip_group_check=True)
                sc = pg.tile([128, 8, C], F32, tag="sc", name="sc", bufs=12)
                nc.scalar.activation(sc[:], ctile[:], AF.Copy)
                so = (q * cfg.QCALLS + g) * 64
                return nc.gpsimd.dma_scatter_add(
                    pq[q][:, :], sc[:], gidx2_sb[:, so:so + 64],
                    1024, 1024, C)

            def emit_dup(q, lv):
                dc = q * cfg.NDUP + lv
                Gd = pg.tile([128, 8, 128], BF16, tag="Gd", name="Gd", bufs=2)
                nc.gpsimd.dma_gather(
                    Gd[:], h2pad[:, :],
                    gidxd_sb[:, dc * 64:(dc + 1) * 64], 1024, 1024, 128)
                scd = pg.tile([128, 8, C], F32, tag="scd", name="scd", bufs=2)
                for cc in range(8):
                    nc.vector.tensor_scalar(
                        scd[:, cc, :], Gd[:, cc, 0:C],
                        gwd_sb[:, dc * 8 + cc:dc * 8 + cc + 1], None, OP.mult)
                return nc.gpsimd.dma_scatter_add(
                    pq[q][:, :], scd[:], sidxd_sb[:, dc * 64:(dc + 1) * 64],
                    1024, 1024, C)

            def emit_rs(q):
                # drain gate: scatter->scatter ordering on one tensor is
                # DMA-completion exact (verified), so a dummy zero scatter
                # into pq[q] cannot start until every real scatter drained;
                # the RS RAW-depends on it.
                gidx_gate = sidxd_sb[:, cfg.NQ * cfg.NDUP * 64:]
                dg = nc.gpsimd.dma_scatter_add(
                    pq[q][:, :], zsrc[:, 0:C].rearrange("p (o f) -> p o f", o=1),
                    gidx_gate, 128, 128, C)
                cc = nc.gpsimd.collective_compute(
                    "ReduceScatter", OP.add,
                    ins=[pq[q][0:cfg.QTOT, :]],
                    outs=[rsq[q][:, :]],
                    replica_groups=[list(range(cfg.NC))],
                )
                bass._add_dep_helper(cc.ins, dg.ins, sync=True,
                                     reason="rs-after-scatter-drain")
                return cc

            def emit_final(q):
                rv = pd.tile([128, 25 * C], F32, tag="rv", name="rv")
                nc.sync.dma_start(
                    out=rv[:],
                    in_=rsq[q][:, :].rearrange("(p i) f -> p (i f)", p=128))
                for i in range(25):
                    w = q * 25 + i
                    xsl = xs_all[:, w * C:(w + 1) * C]
                    nc.vector.tensor_scalar(
                        xsl, h2all[:, w * C:(w + 1) * C],
                        dinv1_sb[:, w:w + 1], None, OP.mult)
                    nc.vector.tensor_add(xsl, xsl, rv[:, i * C:(i + 1) * C])
                    nc.vector.tensor_add(xsl, xsl, b2_sb[:])
                    nm = nm_all[:, w:w + 1]
                    nc.vector.tensor_reduce(nm, xsl, mybir.AxisListType.X,
                                            OP.max, negate=True)
                    es = pd.tile([128, C], F32, tag="es", name="es")
                    nc.scalar.activation(es[:], xsl, AF.Exp, bias=nm,
                                         accum_out=ss_all[:, w:w + 1])
                ls = pc.tile([128, 25], F32, tag=f"ls{q}", name=f"ls{q}")
                nc.scalar.activation(ls[:], ss_all[:, q * 25:(q + 1) * 25],
                                     AF.Ln)
                ost = pd.tile([128, 25 * C], F32, tag="ost", name="ost")
                for i in range(25):
                    w = q * 25 + i
                    nc.vector.tensor_scalar(
                        ost[:, i * C:(i + 1) * C], xs_all[:, w * C:(w + 1) * C],
                        nm_all[:, w:w + 1], ls[:, i:i + 1],
                        OP.add, OP.subtract)
                nc.sync.dma_start(
                    out=outc[q * 128:(q + 1) * 128, :], in_=ost[:])

            # ============== interleaved schedule ==============
            # L1 batch b + quarter-0/1 scatter calls (delayed by DLY batches so
            # the on-device zeroing of pq finishes before the first scatter hits
            # the in-order Pool queue); quarters 2/3 after L1 (overlap RS 0/1);
            # finals pipeline after RS.
            DLY = 8
            # zero the partial tensors up front; SP-issued DMAs serialize with
            # each other but run in parallel with Pool's SWDGE transfers
            zsrc = pc.tile([128, 25 * C], F32, tag="zsrc", name="zsrc")
            nc.vector.memset(zsrc[:], 0.0)
            for q in range(cfg.NQ):
                for g2 in range(8):
                    nc.sync.dma_start(
                        out=pq[q][3200 * g2:3200 * (g2 + 1), :]
                        .rearrange("(p i) f -> p (i f)", p=128),
                        in_=zsrc[:])
            for b in range(NBATCH):
                emit_l1_batch(b)
                if b >= DLY:
                    emit_l2_call(0, b - DLY)
                    emit_l2_call(1, b - DLY)
            for g in range(cfg.QCALLS - DLY, cfg.QCALLS):
                emit_l2_call(0, g)
                emit_l2_call(1, g)
            for lv in range(cfg.NDUP):
                emit_dup(0, lv)
            for lv in range(cfg.NDUP):
                emit_dup(1, lv)
            # alternating tail: RS_q (on the collective cores) overlaps the
            # NEXT quarter's compute; its scatters drain between collectives
            # (a running collective blocks SWDGE DMA).
            # RS_q's dummy-gate scatter is emitted AFTER quarter q+1's calls:
            # the SWDGE serialization then drains q+1's scatters in the CC gap
            # before RS_q starts, so each RS overlaps the next quarter's
            # compute instead of blocking its scatters.
            emit_rs(0)
            for g in range(cfg.QCALLS):
                emit_l2_call(2, g)
            for lv in range(cfg.NDUP):
                emit_dup(2, lv)
            emit_rs(1)
            for g in range(cfg.QCALLS):
                emit_l2_call(3, g)
            for lv in range(cfg.NDUP):
                emit_dup(3, lv)
            emit_final(0)
            emit_rs(2)
            emit_final(1)
            emit_rs(3)
            emit_final(2)
            emit_final(3)
            l2_psum_c.__exit__(None, None, None)
            l2_psum_t.__exit__(None, None, None)
            l1_psum_d.__exit__(None, None, None)
            l1_psum.__exit__(None, None, None)

    nc.compile()
    return nc


_PROGRAM_CACHE = {}


def _get_program(cfg):
    if "prog" not in _PROGRAM_CACHE:
        _PROGRAM_CACHE["prog"] = build_program(cfg)
    return _PROGRAM_CACHE["prog"]


def kernel(x, W1, b1, W2, b2, edge_index):
    cfg = CFG_FULL
    in_maps = prep(x, W1, b1, W2, b2, edge_index, cfg)
    nc = _get_program(cfg)
    res = run_bass_kernel_spmd(
        nc, in_maps, core_ids=list(range(cfg.NC)),
        trace=bool(os.environ.get("GCN_TRACE")))
    if res.exec_time_ns is not None:
        print(f"HW exec time: {res.exec_time_ns} ns")
    out = np.empty((cfg.N, cfg.C), np.float32)
    for k in range(cfg.NC):
        arr = res.results[k]["outc"].reshape(cfg.NQ, 128, 25, cfg.C)
        # node (q, p, i) = k*R + q*3200 + i*128 + p
        blk = arr.transpose(0, 2, 1, 3).reshape(cfg.NQ * 25 * 128, cfg.C)
        n0 = k * cfg.R
        n1 = min(cfg.N, n0 + cfg.R)
        out[n0:n1] = blk[:n1 - n0]
    return out
